# revision 2
# baseline (speedup 1.0000x reference)
"""Trainium2 8-core SPMD kernel for a 3-layer GIN network (GINConv x3 ->
global_add_pool -> Linear -> ReLU -> Linear).

Strategy (graph/edge partition per the sharding hint):
  - 100000 nodes padded to 100352; core c owns 12544 consecutive dst nodes
    (98 blocks of 128); each core processes the edges incident to its dst
    nodes. MLP weights replicated.
  - Node features H stored bf16 [100352, 128] (64 features + pad to 256B
    rows, pad bytes never read).
  - Neighbor gather: dma_gather (custom SWDGE gather ucode) with int16
    in-bank row indices over 4 row-banks of H, one call per
    (superblock, bank) segment to amortize the ~1us SWDGE fixed cost.
    8 msg buffers keep up to 8 superblock gathers in flight ahead of
    compute.
  - The gather stream is 128-aligned per (block, bank) run (padding rows
    gather spread-out rows to avoid same-address HBM channel conflicts,
    and are masked by slot=-1), so every 128-row tile belongs to exactly
    one dst block.
  - Scatter-add per 128-dst block: PSUM[64,128] += msg.T @ M2 on the
    tensor engine, where M2[e,s] = (dstslot[e] == s). M2 is built in
    an [e, s, pair] layout via tensor_tensor(is_equal) against a
    repeated-iota constant so both DVE operands have a packed 2-byte
    innermost dim (2x_1p fast mode). Self loops (z = h + agg) are
    synthetic edges with slot = p.
  - GIN MLPs run in fp32 on the PE (weights fp32; messages bf16).
  - After conv1/conv2: AllGather of the 12544-row shard -> full H.
  - conv3 feeds global_add_pool: PSUM[64,512] += h3.T @ B with B the
    batch one-hot (tensor_scalar per-partition compare vs iota, 4x_2p);
    AllReduce; the final linear head runs fp32 on-core; output
    [512, 1] fp32 taken from core 0.
"""

import numpy as np
import ml_dtypes

import concourse.bass as bass
import concourse.tile as tile
from concourse import bacc, mybir

BF16 = mybir.dt.bfloat16
F16 = mybir.dt.float16
F32 = mybir.dt.float32
I16 = mybir.dt.int16
AF = mybir.ActivationFunctionType
ALU = mybir.AluOpType

D = 64
ROW = 128
CORES = 8
NBANKS = 4
NP_PAD = 100352
N_GRAPHS = 512
N_NODES = 100000
SUPER = 2
GATHER_CHUNK = 2048
NQUEUES = 4


class _Plan:
    pass


QBLK = [0, 25, 50, 74, 98]          # block boundaries of the 4 quarters


def _quarter_layout(NP, cores):
    PER = NP // cores
    qrows = [(QBLK[q + 1] - QBLK[q]) * 128 for q in range(4)]
    bank_starts = [0]
    for q in range(4):
        bank_starts.append(bank_starts[-1] + cores * qrows[q])
    return qrows, bank_starts


def _storage_perm(NP, cores):
    PER = NP // cores
    qrows, bank_starts = _quarter_layout(NP, cores)
    n = np.arange(NP, dtype=np.int64)
    c = n // PER
    r = n % PER
    qb = np.array([0] + [sum(qrows[:q + 1]) for q in range(4)])
    q = np.searchsorted(qb, r, side="right") - 1
    out = np.empty(NP, np.int64)
    for qq in range(4):
        m = q == qq
        out[m] = (bank_starts[qq] + c[m] * qrows[qq] + (r[m] - qb[qq]))
    return out


def _make_plan(src, dst, batch, n_real, NP, S=SUPER, cores=CORES):
    PER = NP // cores
    NBLK = PER // 128
    BANK = NP // NBANKS
    p = _Plan()
    p.NP, p.PER, p.NBLK, p.BANK, p.S, p.cores = NP, PER, NBLK, BANK, S, cores
    P = _storage_perm(NP, cores)
    p.perm = P
    qrows, bank_starts = _quarter_layout(NP, cores)
    p.bank_starts = bank_starts
    bs_arr = np.array(bank_starts, np.int64)

    blk = (dst // 128).astype(np.int64)          # global block id
    slot = (dst % 128).astype(np.int64)
    prow = P[src]
    bank = np.searchsorted(bs_arr, prow, side="right") - 1
    row = (prow - bs_arr[bank]).astype(np.int64)
    core = (dst // PER).astype(np.int64)
    blk_local = blk - core * NBLK

    # per (core, local block, bank) edge counts, incl. 128 self-loop rows
    counts = np.zeros((cores, NBLK, NBANKS), np.int64)
    np.add.at(counts, (core, blk_local, bank), 1)
    self_bank = np.zeros((cores, NBLK), np.int64)
    for c in range(cores):
        for b in range(NBLK):
            pr = P[c * PER + b * 128:c * PER + (b + 1) * 128]
            sb = int(np.searchsorted(bs_arr, pr[0], side="right") - 1)
            assert (np.searchsorted(bs_arr, pr, side="right") - 1 == sb).all()
            self_bank[c, b] = sb
            counts[c, b, sb] += 128
    # aligned run length per (block, bank): max over cores, rounded to 128
    A = ((counts.max(axis=0) + 127) // 128) * 128          # [NBLK, NBANKS]
    T = A // 128

    # edge order: sorted by (global block, bank); per-(c,b,k) contiguous runs
    order = np.lexsort((bank, blk))
    srt_blk = blk[order]; srt_bank = bank[order]
    srt_row = row[order]; srt_slot = slot[order]
    key = srt_blk * NBANKS + srt_bank
    uniq, starts = np.unique(key, return_index=True)
    starts = np.append(starts, len(key))
    seg_of = {int(u): (int(s), int(e)) for u, s, e in zip(uniq, starts[:-1], starts[1:])}

    sb_list = []
    b0 = 0
    while b0 < NBLK:
        sb_list.append((b0, min(b0 + S, NBLK)))
        b0 += S
    p.superblocks = sb_list

    # stream layout: sb-major, bank-minor, block-inner; 128-aligned runs
    gather_cols = []     # per sb: list of (k, ni, col_off, toff)
    sb_tiles = []
    col_base = 0
    p.block_pairs = []   # (b, pair_base, [(tile_in_sb, k), ...])
    pair_base = 0
    tile_total = 0
    blk_pl = {b: [] for b in range(NBLK)}
    for (b0, b1) in sb_list:
        glist = []
        toff = 0
        for k in range(NBANKS):
            ni = int(A[b0:b1, k].sum())
            if ni == 0:
                continue
            glist.append((k, ni, col_base, toff))
            for b in range(b0, b1):
                for t in range(T[b, k]):
                    blk_pl[b].append((toff + t, k))
                toff += T[b, k]
            col_base += ni // 16
        gather_cols.append(glist)
        sb_tiles.append(toff)
        tile_total += toff
    for b in range(NBLK):
        p.block_pairs.append((b, pair_base, blk_pl[b]))
        pair_base += len(blk_pl[b])
    p.gather_cols = gather_cols
    p.sb_tiles = sb_tiles
    p.TS_max = max(sb_tiles)
    p.TT = tile_total
    p.NI_total = tile_total * 128
    p.COLS = col_base
    p.NPAIRS = pair_base
    p.npairs_max = max(len(pl) for (_, _, pl) in p.block_pairs)

    # per-core index stream + slot stream (pad rows: spread idxs, slot -1)
    idx_rows = np.empty((cores, p.NI_total), np.int16)
    _spread = (np.arange(p.NI_total, dtype=np.int64) * 9973) % 24576
    for _c in range(cores):
        idx_rows[_c] = ((_spread + _c * 3037) % 24576).astype(np.int16)
    slot_stream = np.full((cores, p.NI_total), -1.0, np.float32)
    stream_pos = {}      # (sb_idx, k, b) -> stream row offset of the run
    pos0 = 0
    for ib, (b0, b1) in enumerate(sb_list):
        for (k, ni, col_off, toff) in gather_cols[ib]:
            for b in range(b0, b1):
                stream_pos[(ib, k, b)] = pos0
                pos0 += int(A[b, k])
    assert pos0 == p.NI_total

    for c in range(cores):
        for ib, (b0, b1) in enumerate(sb_list):
            for (k, ni, col_off, toff) in gather_cols[ib]:
                for b in range(b0, b1):
                    pos = stream_pos[(ib, k, b)]
                    gblk = c * NBLK + b
                    if self_bank[c, b] == k:
                        nodes = P[np.arange(gblk * 128, gblk * 128 + 128)]
                        idx_rows[c, pos:pos + 128] = (nodes - bank_starts[k]).astype(np.int16)
                        slot_stream[c, pos:pos + 128] = np.arange(128)
                        pos += 128
                    sk = seg_of.get(gblk * NBANKS + k)
                    if sk is not None:
                        s0, s1 = sk
                        idx_rows[c, pos:pos + (s1 - s0)] = srt_row[s0:s1].astype(np.int16)
                        slot_stream[c, pos:pos + (s1 - s0)] = srt_slot[s0:s1]
                        pos += s1 - s0

    # dslot table: per pair, the dst slot of each of its 128 rows (-1 = pad)
    pair_slots = np.full((cores, 128, p.NPAIRS), -1.0, np.float32)
    pair_cursor = {b: 0 for b in range(NBLK)}
    for ib, (b0, b1) in enumerate(sb_list):
        for (k, ni, col_off, toff) in gather_cols[ib]:
            for b in range(b0, b1):
                base = stream_pos[(ib, k, b)]
                _, pb, pl = p.block_pairs[b]
                for t in range(T[b, k]):
                    j = pair_cursor[b]
                    pair_cursor[b] += 1
                    r0 = base + t * 128
                    pair_slots[:, :, pb + j] = slot_stream[:, r0:r0 + 128]
    p.dstslot = pair_slots.astype(ml_dtypes.bfloat16)

    # idx wrapped [128, COLS] int16 per core
    idx_wrapped = np.zeros((cores, 128, p.COLS), np.int16)
    for ib, (b0, b1) in enumerate(sb_list):
        for (k, ni, col_off, toff) in gather_cols[ib]:
            i0 = stream_pos[(ib, k, b0)]
            for c in range(cores):
                w = idx_rows[c, i0:i0 + ni].reshape(ni // 16, 16).T
                idx_wrapped[c, :, col_off:col_off + ni // 16] = np.tile(w, (8, 1))
    p.idx_wrapped = idx_wrapped

    # iota repeated along an npairs_max-wide inner dim: iota_rep[p, s, j] = s
    iota_rep = np.repeat(np.arange(128, dtype=np.float32), p.npairs_max)
    p.iota_rep = np.broadcast_to(
        iota_rep, (128, 128 * p.npairs_max)).astype(ml_dtypes.bfloat16)

    batch_pad = np.full(NP, -1.0, np.float32)
    batch_pad[:n_real] = batch.astype(np.float32)
    batchslot = np.empty((cores, 128, NBLK), np.float32)
    for c in range(cores):
        bs = batch_pad[c * PER:(c + 1) * PER].reshape(NBLK, 128).T
        batchslot[c] = bs.astype(np.float32)
    p.batchslot = batchslot
    return p


def _prep_inputs(p, x, weights):
    NP = p.NP
    h0 = np.zeros((NP, ROW), ml_dtypes.bfloat16)
    h0[p.perm[:x.shape[0]], :D] = x.astype(ml_dtypes.bfloat16)

    iotaG = np.broadcast_to(np.arange(N_GRAPHS, dtype=np.float32), (128, N_GRAPHS)).astype(np.float16)

    shared = {
        "h0": h0,
        "iotarep": np.ascontiguousarray(p.iota_rep),
        "iotag": np.ascontiguousarray(iotaG),
        "ones_row": np.ones((1, 128), np.float32),
    }
    for i in (1, 2, 3):
        shared[f"c{i}w1"] = weights[f"conv{i}_w1"].astype(np.float32)
        shared[f"c{i}b1"] = weights[f"conv{i}_b1"].astype(np.float32).reshape(D, 1)
        shared[f"c{i}w2"] = weights[f"conv{i}_w2"].astype(np.float32)
        shared[f"c{i}b2"] = weights[f"conv{i}_b2"].astype(np.float32).reshape(1, D)
    shared["l1w"] = weights["lin1_w"].astype(np.float32)
    shared["l1b"] = weights["lin1_b"].astype(np.float32).reshape(D, 1)
    shared["l2w"] = weights["lin2_w"].astype(np.float32)
    shared["l2b"] = weights["lin2_b"].astype(np.float32).reshape(1, 1)

    in_maps = []
    for c in range(p.cores):
        m = dict(shared)
        m["idx"] = np.ascontiguousarray(p.idx_wrapped[c])
        m["dslot"] = np.ascontiguousarray(p.dstslot[c])
        m["bslot"] = np.ascontiguousarray(p.batchslot[c])
        in_maps.append(m)
    return in_maps


def _build_nc(p, repeat=1):
    nc = bacc.Bacc("TRN2", target_bir_lowering=False, debug=False,
                   num_devices=p.cores, num_swdge_queues=NQUEUES)
    NP, PER, NBLK, BANK = p.NP, p.PER, p.NBLK, p.BANK
    G = N_GRAPHS
    NPM = p.npairs_max

    h0 = nc.dram_tensor("h0", [NP, ROW], BF16, kind="ExternalInput")
    idx_d = nc.dram_tensor("idx", [128, p.COLS], I16, kind="ExternalInput")
    dslot_d = nc.dram_tensor("dslot", [128, p.NPAIRS], BF16, kind="ExternalInput")
    bslot_d = nc.dram_tensor("bslot", [128, NBLK], F32, kind="ExternalInput")
    iotarep_d = nc.dram_tensor("iotarep", [128, 128 * NPM], BF16, kind="ExternalInput")
    iotag_d = nc.dram_tensor("iotag", [128, G], F16, kind="ExternalInput")
    ones_d = nc.dram_tensor("ones_row", [1, 128], F32, kind="ExternalInput")
    wd = {}
    for i in (1, 2, 3):
        wd[f"c{i}w1"] = nc.dram_tensor(f"c{i}w1", [D, D], F32, kind="ExternalInput")
        wd[f"c{i}b1"] = nc.dram_tensor(f"c{i}b1", [D, 1], F32, kind="ExternalInput")
        wd[f"c{i}w2"] = nc.dram_tensor(f"c{i}w2", [D, D], F32, kind="ExternalInput")
        wd[f"c{i}b2"] = nc.dram_tensor(f"c{i}b2", [1, D], F32, kind="ExternalInput")
    l1w_d = nc.dram_tensor("l1w", [D, D], F32, kind="ExternalInput")
    l1b_d = nc.dram_tensor("l1b", [D, 1], F32, kind="ExternalInput")
    l2w_d = nc.dram_tensor("l2w", [D, 1], F32, kind="ExternalInput")
    l2b_d = nc.dram_tensor("l2b", [1, 1], F32, kind="ExternalInput")
    out_d = nc.dram_tensor("out", [1, G], F32, kind="ExternalOutput")

    rg = [list(range(p.cores))]

    with tile.TileContext(nc) as tc:
        with (
            tc.tile_pool(name="const", bufs=1) as cp,
            tc.tile_pool(name="msg", bufs=8) as msgp,
            tc.tile_pool(name="m2", bufs=4) as m2p,
            tc.tile_pool(name="work", bufs=3) as wp,
            tc.tile_pool(name="hout", bufs=3) as hop,
            tc.tile_pool(name="psA", bufs=2, space="PSUM") as psA,
            tc.tile_pool(name="psB", bufs=3, space="PSUM") as psB,
            tc.tile_pool(name="psC", bufs=2, space="PSUM") as psC,
            tc.tile_pool(name="psPool", bufs=1, space="PSUM") as psP,
            tc.tile_pool(name="dram", bufs=1, space="DRAM") as dp,
        ):
            idx_sb = cp.tile([128, p.COLS], I16)
            nc.sync.dma_start(idx_sb[:], idx_d[:])
            dslot_sb = cp.tile([128, p.NPAIRS], BF16)
            nc.sync.dma_start(dslot_sb[:], dslot_d[:])
            bslot_sb = cp.tile([128, NBLK], F32)
            nc.sync.dma_start(bslot_sb[:], bslot_d[:])
            iotarep = cp.tile([128, 128 * NPM], BF16)
            nc.sync.dma_start(iotarep[:], iotarep_d[:])
            iotag = cp.tile([128, G], F16)
            nc.sync.dma_start(iotag[:], iotag_d[:])
            ones_sb = cp.tile([1, 128], F32)
            nc.sync.dma_start(ones_sb[:], ones_d[:])
            ws = {}
            for i in (1, 2, 3):
                for nm, shape in ((f"c{i}w1", [D, D]), (f"c{i}b1", [D, 1]),
                                  (f"c{i}w2", [D, D]), (f"c{i}b2", [1, D])):
                    ws[nm] = cp.tile(shape, F32, name=nm + "s")
                    nc.sync.dma_start(ws[nm][:], wd[nm][:])
            l1w = cp.tile([D, D], F32)
            nc.sync.dma_start(l1w[:], l1w_d[:])
            l1b = cp.tile([D, 1], F32)
            nc.sync.dma_start(l1b[:], l1b_d[:])
            l2w = cp.tile([D, 1], F32)
            nc.sync.dma_start(l2w[:], l2w_d[:])
            l2b = cp.tile([1, 1], F32)
            nc.sync.dma_start(l2b[:], l2b_d[:])

            h1_loc = dp.tile([PER, ROW], BF16)
            h2_loc = dp.tile([PER, ROW], BF16)
            h1_full = dp.tile([NP, ROW], BF16)
            h2_full = dp.tile([NP, ROW], BF16)
            pool_in = dp.tile([D, G], F32)
            pool_out = dp.tile([D, G], F32)

            pool_ps = psP.tile([D, G], F32, space="PSUM")

            iotarep_v = iotarep[:].rearrange("p (s j) -> p s j", j=NPM)

            gq = [0]

            def conv_layer(li, h_src, h_loc):
                w1, b1 = ws[f"c{li}w1"], ws[f"c{li}b1"]
                w2, b2 = ws[f"c{li}w2"], ws[f"c{li}b2"]
                for ib, (b0, b1blk) in enumerate(p.superblocks):
                    msg = msgp.tile([128, p.TS_max * ROW], BF16, tag="msg")
                    msgv = msg[:].rearrange("p (a b) -> p a b", b=ROW)
                    for (k, ni, col_off, toff) in p.gather_cols[ib]:
                        off = 0
                        while off < ni:
                            cni = min(GATHER_CHUNK, ni - off)
                            nc.gpsimd.dma_gather(
                                out_ap=msgv[:, toff + off // 128:toff + (off + cni) // 128, :],
                                in_ap=h_src[p.bank_starts[k]:p.bank_starts[k + 1], :],
                                idxs_ap=idx_sb[:, col_off + off // 16:col_off + (off + cni) // 16],
                                num_idxs=cni,
                                num_idxs_reg=cni,
                                elem_size=ROW,
                                single_packet=False,
                                queue_num=(ib + k + off // GATHER_CHUNK) % NQUEUES,
                            )
                            gq[0] += 1
                            off += cni
                    for b in range(b0, b1blk):
                        _, pb, pl = p.block_pairs[b]
                        nbp = len(pl)
                        m2 = m2p.tile([128, 128 * NPM], BF16, tag="m2")
                        m2v = m2[:].rearrange("p (s j) -> p s j", j=NPM)
                        nc.vector.tensor_tensor(
                            out=m2v[:, :, 0:nbp],
                            in0=dslot_sb[:, pb:pb + nbp].unsqueeze(1)
                                .broadcast_to((128, 128, nbp)),
                            in1=iotarep_v[:, :, 0:nbp],
                            op=ALU.is_equal,
                        )
                        agg = psA.tile([D, 128], F32, space="PSUM", tag="agg")
                        for j, (t_in_sb, _k) in enumerate(pl):
                            nc.tensor.matmul(
                                agg[:], lhsT=msgv[:, t_in_sb, 0:D],
                                rhs=m2v[:, :, j],
                                start=(j == 0), stop=(j == nbp - 1),
                            )
                        z = wp.tile([D, 128], F32, tag="z")
                        nc.scalar.activation(z[:], agg[:], AF.Copy)
                        ps1 = psB.tile([D, 128], F32, space="PSUM", tag="mlp1")
                        nc.tensor.matmul(ps1[:], lhsT=w1[:], rhs=z[:],
                                         start=True, stop=True)
                        a1 = wp.tile([D, 128], F32, tag="a1")
                        nc.scalar.activation(a1[:], ps1[:], AF.Relu, bias=b1[:])
                        ps2 = psC.tile([128, D], F32, space="PSUM", tag="mlp2")
                        nc.tensor.matmul(ps2[:], lhsT=a1[:], rhs=w2[:],
                                         start=True, stop=False)
                        nc.tensor.matmul(ps2[:], lhsT=ones_sb[:], rhs=b2[:],
                                         start=False, stop=True)
                        h3 = hop.tile([128, D], BF16, tag="h3")
                        nc.scalar.activation(h3[:], ps2[:], AF.Relu)
                        if h_loc is not None:
                            nc.sync.dma_start(h_loc[b * 128:(b + 1) * 128, 0:D], h3[:])
                        else:
                            B = wp.tile([128, G], F16, tag="bsel")
                            nc.vector.tensor_scalar(
                                out=B[:],
                                in0=iotag[:],
                                scalar1=bslot_sb[:, b:b + 1],
                                scalar2=None,
                                op0=ALU.is_equal,
                            )
                            nc.tensor.matmul(pool_ps[:], lhsT=h3[:], rhs=B[:],
                                             start=(b == 0), stop=(b == NBLK - 1),
                                             skip_group_check=True)

            for _rep in range(repeat):
                bs = p.bank_starts
                lq = [q * 128 for q in QBLK]     # local row boundaries

                def split_ag(h_loc_t, h_full_t):
                    for q in range(4):
                        nc.gpsimd.collective_compute(
                            "AllGather", ALU.bypass, replica_groups=rg,
                            ins=[h_loc_t[:][lq[q]:lq[q + 1], :]],
                            outs=[h_full_t[:][bs[q]:bs[q + 1], :]])

                conv_layer(1, h0[:], h1_loc[:])
                split_ag(h1_loc, h1_full)
                conv_layer(2, h1_full[:], h2_loc[:])
                split_ag(h2_loc, h2_full)
                conv_layer(3, h2_full[:], None)

                pool_sb = wp.tile([D, G], F32, tag="poolsb")
                nc.vector.tensor_copy(pool_sb[:], pool_ps[:])
                nc.sync.dma_start(pool_in[:], pool_sb[:])
                nc.gpsimd.collective_compute(
                    "AllReduce", ALU.add, replica_groups=rg,
                    ins=[pool_in.opt()], outs=[pool_out.opt()])
                pool_red = wp.tile([D, G], F32, tag="poolred")
                nc.sync.dma_start(pool_red[:], pool_out[:])
                ph1 = psB.tile([D, G], F32, space="PSUM", tag="mlp1")
                nc.tensor.matmul(ph1[:], lhsT=l1w[:], rhs=pool_red[:],
                                 start=True, stop=True)
                s1 = wp.tile([D, G], F32, tag="s1")
                nc.scalar.activation(s1[:], ph1[:], AF.Relu, bias=l1b[:])
                ph2 = psC.tile([1, G], F32, space="PSUM", tag="mlp2")
                nc.tensor.matmul(ph2[:], lhsT=l2w[:], rhs=s1[:],
                                 start=True, stop=True)
                og = wp.tile([1, G], F32, tag="og")
                nc.scalar.activation(og[:], ph2[:], AF.Identity, bias=l2b[:])
                nc.sync.dma_start(out_d[:], og[:])

    nc.compile()
    return nc


def kernel(**inputs):
    x = np.asarray(inputs["x"], np.float32)
    ei = np.asarray(inputs["edge_index"], np.int64)
    src, dst = ei[0], ei[1]
    batch = np.asarray(inputs["batch"], np.int64)
    weights = {k: np.asarray(v, np.float32) for k, v in inputs.items()
               if k not in ("x", "edge_index", "batch")}

    p = _make_plan(src, dst, batch, x.shape[0], NP_PAD)
    in_maps = _prep_inputs(p, x, weights)
    nc = _build_nc(p)

    from concourse import bass_utils
    res = bass_utils.run_bass_kernel_spmd(nc, in_maps, core_ids=list(range(CORES)))
    out = res.results[0]["out"]
    return out.reshape(-1)[:N_GRAPHS].reshape(N_GRAPHS, 1).astype(np.float32)

